# revision 53
# baseline (speedup 1.0000x reference)
"""Trainium2 Bass kernel for nn_ContactPredictionHead.

Reference computation (B=2, L=2048, D=1536, T=2):
    Wp, Wd = W[:, :D], W[:, D:]
    prod[b,i,j,t] = sum_d h[b,i,d] * Wp[t,d] * h[b,j,d]
    diff[b,i,j,t] = (h@Wd.T)[b,i,t] - (h@Wd.T)[b,j,t]
    out = symmetrize(prod + diff + bias)

Key identity: prod is symmetric in (i,j) and diff is antisymmetric, so the
symmetrization leaves   out[b,i,j,t] = prod[b,i,j,t] + bias[t]   exactly —
a weighted Gram matrix.  Only the upper triangle is computed on device; the
host mirrors the strict lower triangle.

Sharding: 4 cores per batch item.  The 16 row-blocks (128 rows each) of a
batch's L x L Gram matrix are dealt by a Latin square: core cc's stationary
slot s holds row-block I = 4s + ((s+cc)%4).  Slot s covers its arc
[128I, 2048) as one cc-dependent "partial" group [128I, 512(s+1)) plus
(3-s) full 512-col "static" groups — so every core computes exactly
4352 moving columns per t (the balanced ideal; the aligned scheme needs
5120).  Static groups are identical on all cores; the partial groups'
offsets/sizes live in a 4-way partition-id branch on the Tensor engine
only (all other engines run straight-line code: PSUM accs are padded to
512 and the host slices each group's valid columns).

Phases run in REVERSE chunk order (3,2,1,0): per-phase matmul work
(18.6/13.4/8.3/3.2 us) then always exceeds the per-chunk stream time, so
the PE can never starve on the input stream — no mid-run clock throttle,
robust to HBM contention.  This requires the stationary windows to arrive
independently of the chunks: the host gathers each core's four 128-row
slot windows into a separate small input ("hw", 0.4MB/slot), which also
makes the weight-multiply preps straight-line (per-core-ness lives in the
data, not the addressing).

All tensors stream as bfloat16 (PSUM accumulates fp32), halving DMA
volume vs fp32 at the same PE rate.
"""
import sys

sys.path.insert(0, "/opt/trn_rl_repo")

import numpy as np
import ml_dtypes

BF16 = ml_dtypes.bfloat16

B, L, D, T = 2, 2048, 1536, 2
NCORES = 8
CPB = NCORES // B     # cores per batch item = 4
NK = D // 128         # contraction k-tiles = 12
NJ = 512              # j columns per full matmul (one PSUM bank of fp32)
NNB = L // NJ         # j chunks = 4
NS = 4                # stationary row slots per core (128 rows each)

PHASES = [3, 2, 1, 0]           # chunk/phase order (heaviest work first)
PREPS = [0, 1, 2, 3]            # slot-window arrival + prep order
# DMA k-parts per chunk: chunk 3 (consumed first) is quartered so the
# first matmuls start on a quarter-chunk; finer splits lose more to the
# ~0.7us fixed issue cost per DMA than the earlier starts gain.
PARTS = {3: 4, 2: 2, 1: 2, 0: 2}


def row_of(s, cc):
    """Global 128-row block held by slot s on a core with variant cc."""
    return 4 * s + (s + cc) % 4


def groups_of(cc):
    """Schedule (shared shape, variant-dependent geometry): list of
    (I, colstart, F) in emission order; 20 groups = [statics (s<v) t0, t1,
    then partial t0, t1] per chunk phase v, phases in PHASES order.  The
    partial goes last: the statics only need the early slot windows, so
    the phase's bulk is never gated on the late-arriving window of its own
    slot."""
    gs = []
    for v in PHASES:
        q = (v + cc) % 4
        for s in range(v):
            for _t in range(T):
                gs.append((row_of(s, cc), NJ * v, NJ))
        for _t in range(T):
            gs.append((row_of(v, cc), NJ * v + 128 * q, NJ - 128 * q))
    return gs


NG = len(groups_of(0))    # 20

_CACHE = {}


def _get_nc():
    if "nc" in _CACHE:
        return _CACHE["nc"]
    import concourse.tile as tile
    from concourse.tile_rust import add_dep_helper
    from concourse import bacc, mybir

    f32, bf16 = mybir.dt.float32, mybir.dt.bfloat16
    nc = bacc.Bacc("TRN2", target_bir_lowering=False, debug=False,
                   num_devices=NCORES, enable_partition_id=True,
                   enable_asserts=False)
    ht_d = nc.dram_tensor("ht", [D, L], bf16, kind="ExternalInput")
    hw_d = nc.dram_tensor("hw", [D, NS * 128], bf16, kind="ExternalInput")
    wp_d = nc.dram_tensor("wp", [128, T * NK], bf16, kind="ExternalInput")
    out_d = nc.dram_tensor("out", [NG, 128, NJ], bf16, kind="ExternalOutput")

    with tile.TileContext(nc) as tc:
        with tc.tile_pool(name="big", bufs=1) as big, \
             tc.tile_pool(name="st", bufs=4) as stp, \
             tc.tile_pool(name="ps", bufs=4, space="PSUM") as psp, \
             tc.tile_pool(name="psw", bufs=1, space="PSUM") as psw:
            # wt[p, t*NK+k] = Wp[t, 128k+p] (pre-gathered on the host)
            wt = big.tile([128, T * NK], bf16, name="wt")
            # hst[p, s, t, k, r] = hw[128k+p, 128s+r] * Wp[t, 128k+p]
            hst = big.tile([128, NS, T, NK, 128], bf16, name="hst")
            # htw[p, k, 128s+r] = hw[128k+p, 128s+r]  (slot windows)
            htw = big.tile([128, NK, NS * 128], bf16, name="htw")
            # htall[p, k, j] = ht[128k+p, j]  (canonical, un-rolled)
            htall = big.tile([128, NK, L], bf16, name="htall")

            # Partition-id register load costs ~1.4us of queue time — issue
            # it first so it overlaps the framework preamble and DMA issues
            # instead of delaying the branch evaluations later.
            pid = nc.tensor.partition_id()
            cc = pid % 4

            nc.scalar.dma_start(wt[:], wp_d.ap())
            # Warm the PE clock (HAM un-throttles after ~3.4 us of activity)
            # with throwaway matmuls on a locally-initialized scratch tile —
            # no DMA dependency, so warmup starts during the preamble.  The
            # memset rides gpsimd so the vector queue stays clear for the
            # stationary preps.
            wdum = big.tile([128, NJ], bf16, name="wdum")
            nc.gpsimd.memset(wdum[:], 0.0)
            wacc = psw.tile([128, NJ], f32, name="wacc")
            for _ in range(16):
                nc.tensor.matmul(wacc[:, 0:128], wdum[:, 0:128],
                                 wdum[:, 0:128], start=True, stop=True)

            # Input stream, strictly ordered on the sync ring in consumption
            # order: slot-3 window, chunk 3, remaining slot windows, then
            # chunks 2, 1, 0.
            prev = None

            def chain(dma):
                nonlocal prev
                if prev is not None:
                    add_dep_helper(dma.ins, prev.ins, sync=False,
                                   reason="input stream in consumption order")
                prev = dma

            def win_dma(s):
                chain(nc.sync.dma_start(
                    htw[:, :, 128 * s:128 * (s + 1)],
                    hw_d.ap()[:, 128 * s:128 * (s + 1)]
                    .rearrange("(k p) r -> p k r", p=128)))

            def chunk_part(v, h, kq):
                chain(nc.sync.dma_start(
                    htall[:, h * kq:(h + 1) * kq, v * NJ:(v + 1) * NJ],
                    ht_d.ap()[h * kq * 128:(h + 1) * kq * 128,
                              v * NJ:(v + 1) * NJ]
                    .rearrange("(k p) j -> p k j", p=128)))

            # Slot-0 window first (gates the first static matmul), then all
            # of chunk 3 (it paces the whole first phase), then the
            # remaining slot windows (consumed in s order; slot v's own
            # window isn't needed until the end of phase v), then chunks
            # 2, 1, 0.
            win_dma(PREPS[0])
            for h in range(PARTS[3]):
                chunk_part(3, h, NK // PARTS[3])
            for s in PREPS[1:]:
                win_dma(s)
            for v in PHASES[1:]:
                for h in range(PARTS[v]):
                    chunk_part(v, h, NK // PARTS[v])

            # Stationary prep (straight-line; per-core rows arrive via hw):
            # hst[:, s, t] = htw window s  *  Wp[t] broadcast along rows.
            for s in PREPS:
                for t in range(T):
                    scale = (wt[:, t * NK:(t + 1) * NK].unsqueeze(2)
                             .broadcast_to([128, NK, 128]))
                    nc.vector.tensor_mul(
                        hst[:, s, t], htw[:, :, 128 * s:128 * (s + 1)],
                        scale)

            def emit_partial(v, accs, ccv):
                q = (v + ccv) % 4
                off, fw = NJ * v + 128 * q, NJ - 128 * q
                for t in range(T):
                    for k in range(NK):
                        nc.tensor.matmul(
                            accs[t][:, 0:fw], hst[:, v, t, k],
                            htall[:, k, off:off + fw],
                            start=(k == 0), stop=(k == NK - 1))

            def emit_static(s, v, acc, t):
                for k in range(NK):
                    nc.tensor.matmul(
                        acc[:], hst[:, s, t, k],
                        htall[:, k, v * NJ:(v + 1) * NJ],
                        start=(k == 0), stop=(k == NK - 1))

            def store(gi, acc):
                # Copies on scalar: the vector queue must stay free for the
                # stationary preps (a copy would trap a later prep behind a
                # matmul completion).  The last two groups (the v=0
                # partials) route around gpsimd, whose teardown drain is
                # ~2.7us: t0's chain hides inside t1's matmuls; t1's store
                # splits into two halves on parallel vector+sync /
                # scalar+scalar chains (sync's drain is ~8ns).
                if gi == NG - 2:
                    st = stp.tile([128, NJ], bf16, name="st", tag="st")
                    nc.vector.tensor_copy(st[:], acc[:])
                    nc.sync.dma_start(out_d.ap()[gi], st[:])
                elif gi == NG - 1:
                    for hh in range(2):
                        sth = stp.tile([128, NJ // 2], bf16, name="sth",
                                       tag="sth")
                        half = slice(hh * (NJ // 2), (hh + 1) * (NJ // 2))
                        if hh == 0:
                            nc.scalar.copy(sth[:], acc[:, half])
                            nc.scalar.dma_start(out_d.ap()[gi][:, half],
                                                sth[:])
                        else:
                            nc.vector.tensor_copy(sth[:], acc[:, half])
                            nc.sync.dma_start(out_d.ap()[gi][:, half],
                                              sth[:])
                else:
                    st = stp.tile([128, NJ], bf16, name="st", tag="st")
                    nc.scalar.copy(st[:], acc[:])
                    nc.gpsimd.dma_start(out_d.ap()[gi], st[:])

            gi = 0
            for v in PHASES:
                for s in range(v):
                    for t in range(T):
                        acc = psp.tile([128, NJ], f32, name="acc", tag="acc")
                        emit_static(s, v, acc, t)
                        store(gi, acc)
                        gi += 1
                accs = [psp.tile([128, NJ], f32, name="acc", tag="acc")
                        for _t in range(T)]
                with tc.If(cc <= 1) as c1:
                    with tc.If(cc == 0) as c2:
                        emit_partial(v, accs, 0)
                    with c2.Else():
                        emit_partial(v, accs, 1)
                with c1.Else():
                    with tc.If(cc == 2) as c3:
                        emit_partial(v, accs, 2)
                    with c3.Else():
                        emit_partial(v, accs, 3)
                for t in range(T):
                    store(gi, accs[t])
                    gi += 1
    nc.compile()
    _CACHE["nc"] = nc
    return nc


def make_in_maps(h, W):
    # wp[p, t*NK+k] = Wp[t, 128k+p]
    wp = np.ascontiguousarray(
        W[:, :D].reshape(T, NK, 128).transpose(2, 0, 1)
        .reshape(128, T * NK)).astype(BF16)
    hts = [np.ascontiguousarray(h[bi].T).astype(BF16) for bi in range(B)]
    in_maps = []
    for c in range(NCORES):
        bi, cc = c // CPB, c % CPB
        hw = np.concatenate(
            [hts[bi][:, 128 * row_of(s, cc):128 * row_of(s, cc) + 128]
             for s in range(NS)], axis=1)
        in_maps.append({"ht": hts[bi], "hw": np.ascontiguousarray(hw),
                        "wp": wp})
    return in_maps


def kernel(hidden_states, W, b):
    from concourse.bass_utils import run_bass_kernel_spmd

    h = np.ascontiguousarray(hidden_states, dtype=np.float32)
    W = np.asarray(W, dtype=np.float32)
    bias = np.asarray(b, dtype=np.float32)
    nc = _get_nc()

    res = run_bass_kernel_spmd(nc, make_in_maps(h, W),
                               core_ids=list(range(NCORES)))
    full = np.empty((B, L, L, T), np.float32)
    for c in range(NCORES):
        bi, cc = c // CPB, c % CPB
        blocks = np.asarray(res.results[c]["out"]).astype(np.float32)
        for gi, (I, colstart, fw) in enumerate(groups_of(cc)):
            t = gi % T
            rows = slice(128 * I, 128 * I + 128)
            full[bi, rows, colstart:colstart + fw, t] = blocks[gi, :, 0:fw]
    # Mirror: keep computed j >= i, take j < i from the transpose.
    idx = np.arange(L)
    mask = (idx[None, :] >= idx[:, None])[None, :, :, None]
    out = np.where(mask, full, full.transpose(0, 2, 1, 3))
    if np.any(bias != 0):
        out += bias
    return out


# revision 54
# speedup vs baseline: 1.1656x; 1.1656x over previous
"""Trainium2 Bass kernel for nn_ContactPredictionHead.

Reference computation (B=2, L=2048, D=1536, T=2):
    Wp, Wd = W[:, :D], W[:, D:]
    prod[b,i,j,t] = sum_d h[b,i,d] * Wp[t,d] * h[b,j,d]
    diff[b,i,j,t] = (h@Wd.T)[b,i,t] - (h@Wd.T)[b,j,t]
    out = symmetrize(prod + diff + bias)

Key identity: prod is symmetric in (i,j) and diff is antisymmetric, so the
symmetrization leaves   out[b,i,j,t] = prod[b,i,j,t] + bias[t]   exactly —
a weighted Gram matrix.  Only the upper triangle is computed on device; the
host mirrors the strict lower triangle.

Sharding: 4 cores per batch item.  The 16 row-blocks (128 rows each) of a
batch's L x L Gram matrix are dealt by a Latin square: core cc's stationary
slot s holds row-block I = 4s + ((s+cc)%4).  Slot s covers its arc
[128I, 2048) as one cc-dependent "partial" group [128I, 512(s+1)) plus
(3-s) full 512-col "static" groups — so every core computes exactly
4352 moving columns per t (the balanced ideal; the aligned scheme needs
5120).  Static groups are identical on all cores; the partial groups'
offsets/sizes live in a 4-way partition-id branch on the Tensor engine
only (all other engines run straight-line code: PSUM accs are padded to
512 and the host slices each group's valid columns).

Phases run in REVERSE chunk order (3,2,1,0): per-phase matmul work
(18.6/13.4/8.3/3.2 us) then always exceeds the per-chunk stream time, so
the PE can never starve on the input stream — no mid-run clock throttle,
robust to HBM contention.  This requires the stationary windows to arrive
independently of the chunks: the host gathers each core's four 128-row
slot windows into a separate small input ("hw", 0.4MB/slot), which also
makes the weight-multiply preps straight-line (per-core-ness lives in the
data, not the addressing).

All tensors stream as bfloat16 (PSUM accumulates fp32), halving DMA
volume vs fp32 at the same PE rate.
"""
import sys

sys.path.insert(0, "/opt/trn_rl_repo")

import numpy as np
import ml_dtypes

BF16 = ml_dtypes.bfloat16

B, L, D, T = 2, 2048, 1536, 2
NCORES = 8
CPB = NCORES // B     # cores per batch item = 4
NK = D // 128         # contraction k-tiles = 12
NJ = 512              # j columns per full matmul (one PSUM bank of fp32)
NNB = L // NJ         # j chunks = 4
NS = 4                # stationary row slots per core (128 rows each)

PHASES = [3, 2, 1, 0]           # chunk/phase order (heaviest work first)
PREPS = [0, 1, 2, 3]            # slot-window arrival + prep order
# DMA k-parts per chunk: chunk 3 (consumed first) is quartered so the
# first matmuls start on a quarter-chunk; finer splits lose more to the
# ~0.7us fixed issue cost per DMA than the earlier starts gain.
PARTS = {3: 4, 2: 2, 1: 2, 0: 2}


def row_of(s, cc):
    """Global 128-row block held by slot s on a core with variant cc."""
    return 4 * s + (s + cc) % 4


def groups_of(cc):
    """Schedule (shared shape, variant-dependent geometry): list of
    (I, colstart, F) in emission order; 20 groups = [statics (s<v) t0, t1,
    then partial t0, t1] per chunk phase v, phases in PHASES order.  The
    partial goes last: the statics only need the early slot windows, so
    the phase's bulk is never gated on the late-arriving window of its own
    slot."""
    gs = []
    for v in PHASES:
        q = (v + cc) % 4
        for s in range(v):
            for _t in range(T):
                gs.append((row_of(s, cc), NJ * v, NJ))
        for _t in range(T):
            gs.append((row_of(v, cc), NJ * v + 128 * q, NJ - 128 * q))
    return gs


NG = len(groups_of(0))    # 20

_CACHE = {}


def _get_nc():
    if "nc" in _CACHE:
        return _CACHE["nc"]
    import concourse.tile as tile
    from concourse.tile_rust import add_dep_helper
    from concourse import bacc, mybir

    f32, bf16 = mybir.dt.float32, mybir.dt.bfloat16
    nc = bacc.Bacc("TRN2", target_bir_lowering=False, debug=False,
                   num_devices=NCORES, enable_partition_id=True,
                   enable_asserts=False)
    ht_d = nc.dram_tensor("ht", [D, L], bf16, kind="ExternalInput")
    hw_d = nc.dram_tensor("hw", [D, NS * 128], bf16, kind="ExternalInput")
    wp_d = nc.dram_tensor("wp", [128, T * NK], bf16, kind="ExternalInput")
    out_d = nc.dram_tensor("out", [NG, 128, NJ], bf16, kind="ExternalOutput")

    with tile.TileContext(nc) as tc:
        with tc.tile_pool(name="big", bufs=1) as big, \
             tc.tile_pool(name="st", bufs=4) as stp, \
             tc.tile_pool(name="ps", bufs=4, space="PSUM") as psp, \
             tc.tile_pool(name="psw", bufs=1, space="PSUM") as psw:
            # wt[p, t*NK+k] = Wp[t, 128k+p] (pre-gathered on the host)
            wt = big.tile([128, T * NK], bf16, name="wt")
            # hst[p, s, t, k, r] = hw[128k+p, 128s+r] * Wp[t, 128k+p]
            hst = big.tile([128, NS, T, NK, 128], bf16, name="hst")
            # htw[p, k, 128s+r] = hw[128k+p, 128s+r]  (slot windows)
            htw = big.tile([128, NK, NS * 128], bf16, name="htw")
            # htall[p, k, j] = ht[128k+p, j]  (canonical, un-rolled)
            htall = big.tile([128, NK, L], bf16, name="htall")

            # Partition-id register load costs ~1.4us of queue time — issue
            # it first so it overlaps the framework preamble and DMA issues
            # instead of delaying the branch evaluations later.
            pid = nc.tensor.partition_id()
            cc = pid % 4

            nc.scalar.dma_start(wt[:], wp_d.ap())
            # Warm the PE clock (HAM un-throttles after ~3.4 us of activity)
            # with throwaway matmuls on a locally-initialized scratch tile —
            # no DMA dependency, so warmup starts during the preamble.  The
            # memset rides gpsimd so the vector queue stays clear for the
            # stationary preps.
            wdum = big.tile([128, NJ], bf16, name="wdum")
            nc.gpsimd.memset(wdum[:], 0.0)
            wacc = psw.tile([128, NJ], f32, name="wacc")
            for _ in range(16):
                nc.tensor.matmul(wacc[:, 0:128], wdum[:, 0:128],
                                 wdum[:, 0:128], start=True, stop=True)

            # Input stream, strictly ordered on the sync ring in consumption
            # order: slot-3 window, chunk 3, remaining slot windows, then
            # chunks 2, 1, 0.
            prev = None

            def chain(dma):
                nonlocal prev
                if prev is not None:
                    add_dep_helper(dma.ins, prev.ins, sync=False,
                                   reason="input stream in consumption order")
                prev = dma

            def win_dma(s):
                chain(nc.sync.dma_start(
                    htw[:, :, 128 * s:128 * (s + 1)],
                    hw_d.ap()[:, 128 * s:128 * (s + 1)]
                    .rearrange("(k p) r -> p k r", p=128)))

            def chunk_part(v, h, kq):
                chain(nc.sync.dma_start(
                    htall[:, h * kq:(h + 1) * kq, v * NJ:(v + 1) * NJ],
                    ht_d.ap()[h * kq * 128:(h + 1) * kq * 128,
                              v * NJ:(v + 1) * NJ]
                    .rearrange("(k p) j -> p k j", p=128)))

            # Slot-0 window first (gates the first static matmul), then all
            # of chunk 3 (it paces the whole first phase), then the
            # remaining slot windows (consumed in s order; slot v's own
            # window isn't needed until the end of phase v), then chunks
            # 2, 1, 0.
            win_dma(PREPS[0])
            for h in range(PARTS[3]):
                chunk_part(3, h, NK // PARTS[3])
            for s in PREPS[1:]:
                win_dma(s)
            for v in PHASES[1:]:
                for h in range(PARTS[v]):
                    chunk_part(v, h, NK // PARTS[v])

            # Stationary prep (straight-line; per-core rows arrive via hw):
            # hst[:, s, t] = htw window s  *  Wp[t] broadcast along rows.
            for s in PREPS:
                for t in range(T):
                    scale = (wt[:, t * NK:(t + 1) * NK].unsqueeze(2)
                             .broadcast_to([128, NK, 128]))
                    nc.vector.tensor_mul(
                        hst[:, s, t], htw[:, :, 128 * s:128 * (s + 1)],
                        scale)

            def emit_partial(v, accs, ccv):
                q = (v + ccv) % 4
                off, fw = NJ * v + 128 * q, NJ - 128 * q
                for t in range(T):
                    for k in range(NK):
                        nc.tensor.matmul(
                            accs[t][:, 0:fw], hst[:, v, t, k],
                            htall[:, k, off:off + fw],
                            start=(k == 0), stop=(k == NK - 1))

            def emit_static(s, v, acc, t):
                for k in range(NK):
                    nc.tensor.matmul(
                        acc[:], hst[:, s, t, k],
                        htall[:, k, v * NJ:(v + 1) * NJ],
                        start=(k == 0), stop=(k == NK - 1))

            def store(gi, acc):
                # Copies on scalar: the vector queue must stay free for the
                # stationary preps (a copy would trap a later prep behind a
                # matmul completion).  The last two groups (the v=0
                # partials) route around gpsimd, whose teardown drain is
                # ~2.7us: t0's chain hides inside t1's matmuls; t1's store
                # splits into two halves on parallel vector+sync /
                # scalar+scalar chains (sync's drain is ~8ns).
                if gi == NG - 2:
                    st = stp.tile([128, NJ], bf16, name="st", tag="st")
                    nc.vector.tensor_copy(st[:], acc[:])
                    nc.sync.dma_start(out_d.ap()[gi], st[:])
                elif gi == NG - 1:
                    for hh in range(2):
                        sth = stp.tile([128, NJ // 2], bf16, name="sth",
                                       tag="sth")
                        half = slice(hh * (NJ // 2), (hh + 1) * (NJ // 2))
                        if hh == 0:
                            nc.scalar.copy(sth[:], acc[:, half])
                            nc.scalar.dma_start(out_d.ap()[gi][:, half],
                                                sth[:])
                        else:
                            nc.vector.tensor_copy(sth[:], acc[:, half])
                            nc.sync.dma_start(out_d.ap()[gi][:, half],
                                              sth[:])
                else:
                    st = stp.tile([128, NJ], bf16, name="st", tag="st")
                    nc.scalar.copy(st[:], acc[:])
                    nc.gpsimd.dma_start(out_d.ap()[gi], st[:])

            gi = 0
            for v in PHASES:
                first_s = 0
                if v == PHASES[0] and v > 0:
                    # The opening static pair is paced by the chunk-3
                    # stream: one group consumes a k-quarter in ~0.6us but
                    # quarters arrive ~1.1us apart.  Interleaving t0/t1 by
                    # quarters (two PSUM banks accumulating concurrently)
                    # matches consumption to arrival and closes the
                    # per-quarter gaps.
                    accs0 = [psp.tile([128, NJ], f32, name="acc", tag="acc")
                             for _t in range(T)]
                    kq = NK // PARTS[v]
                    for h in range(PARTS[v]):
                        for t in range(T):
                            for k in range(h * kq, (h + 1) * kq):
                                nc.tensor.matmul(
                                    accs0[t][:], hst[:, 0, t, k],
                                    htall[:, k, v * NJ:(v + 1) * NJ],
                                    start=(k == 0), stop=(k == NK - 1))
                    for t in range(T):
                        store(gi, accs0[t])
                        gi += 1
                    first_s = 1
                for s in range(first_s, v):
                    for t in range(T):
                        acc = psp.tile([128, NJ], f32, name="acc", tag="acc")
                        emit_static(s, v, acc, t)
                        store(gi, acc)
                        gi += 1
                accs = [psp.tile([128, NJ], f32, name="acc", tag="acc")
                        for _t in range(T)]
                with tc.If(cc <= 1) as c1:
                    with tc.If(cc == 0) as c2:
                        emit_partial(v, accs, 0)
                    with c2.Else():
                        emit_partial(v, accs, 1)
                with c1.Else():
                    with tc.If(cc == 2) as c3:
                        emit_partial(v, accs, 2)
                    with c3.Else():
                        emit_partial(v, accs, 3)
                for t in range(T):
                    store(gi, accs[t])
                    gi += 1
    nc.compile()
    _CACHE["nc"] = nc
    return nc


def make_in_maps(h, W):
    # wp[p, t*NK+k] = Wp[t, 128k+p]
    wp = np.ascontiguousarray(
        W[:, :D].reshape(T, NK, 128).transpose(2, 0, 1)
        .reshape(128, T * NK)).astype(BF16)
    hts = [np.ascontiguousarray(h[bi].T).astype(BF16) for bi in range(B)]
    in_maps = []
    for c in range(NCORES):
        bi, cc = c // CPB, c % CPB
        hw = np.concatenate(
            [hts[bi][:, 128 * row_of(s, cc):128 * row_of(s, cc) + 128]
             for s in range(NS)], axis=1)
        in_maps.append({"ht": hts[bi], "hw": np.ascontiguousarray(hw),
                        "wp": wp})
    return in_maps


def kernel(hidden_states, W, b):
    from concourse.bass_utils import run_bass_kernel_spmd

    h = np.ascontiguousarray(hidden_states, dtype=np.float32)
    W = np.asarray(W, dtype=np.float32)
    bias = np.asarray(b, dtype=np.float32)
    nc = _get_nc()

    res = run_bass_kernel_spmd(nc, make_in_maps(h, W),
                               core_ids=list(range(NCORES)))
    full = np.empty((B, L, L, T), np.float32)
    for c in range(NCORES):
        bi, cc = c // CPB, c % CPB
        blocks = np.asarray(res.results[c]["out"]).astype(np.float32)
        for gi, (I, colstart, fw) in enumerate(groups_of(cc)):
            t = gi % T
            rows = slice(128 * I, 128 * I + 128)
            full[bi, rows, colstart:colstart + fw, t] = blocks[gi, :, 0:fw]
    # Mirror: keep computed j >= i, take j < i from the transpose.
    idx = np.arange(L)
    mask = (idx[None, :] >= idx[:, None])[None, :, :, None]
    out = np.where(mask, full, full.transpose(0, 2, 1, 3))
    if np.any(bias != 0):
        out += bias
    return out


# revision 55
# speedup vs baseline: 1.1862x; 1.0177x over previous
"""Trainium2 Bass kernel for nn_ContactPredictionHead.

Reference computation (B=2, L=2048, D=1536, T=2):
    Wp, Wd = W[:, :D], W[:, D:]
    prod[b,i,j,t] = sum_d h[b,i,d] * Wp[t,d] * h[b,j,d]
    diff[b,i,j,t] = (h@Wd.T)[b,i,t] - (h@Wd.T)[b,j,t]
    out = symmetrize(prod + diff + bias)

Key identity: prod is symmetric in (i,j) and diff is antisymmetric, so the
symmetrization leaves   out[b,i,j,t] = prod[b,i,j,t] + bias[t]   exactly —
a weighted Gram matrix.  Only the upper triangle is computed on device; the
host mirrors the strict lower triangle.

Sharding: 4 cores per batch item.  The 16 row-blocks (128 rows each) of a
batch's L x L Gram matrix are dealt by a Latin square: core cc's stationary
slot s holds row-block I = 4s + ((s+cc)%4).  Slot s covers its arc
[128I, 2048) as one cc-dependent "partial" group [128I, 512(s+1)) plus
(3-s) full 512-col "static" groups — so every core computes exactly
4352 moving columns per t (the balanced ideal; the aligned scheme needs
5120).  Static groups are identical on all cores; the partial groups'
offsets/sizes live in a 4-way partition-id branch on the Tensor engine
only (all other engines run straight-line code: PSUM accs are padded to
512 and the host slices each group's valid columns).

Phases run in REVERSE chunk order (3,2,1,0): per-phase matmul work
(18.6/13.4/8.3/3.2 us) then always exceeds the per-chunk stream time, so
the PE can never starve on the input stream — no mid-run clock throttle,
robust to HBM contention.  This requires the stationary windows to arrive
independently of the chunks: the host gathers each core's four 128-row
slot windows into a separate small input ("hw", 0.4MB/slot), which also
makes the weight-multiply preps straight-line (per-core-ness lives in the
data, not the addressing).

All tensors stream as bfloat16 (PSUM accumulates fp32), halving DMA
volume vs fp32 at the same PE rate.
"""
import sys

sys.path.insert(0, "/opt/trn_rl_repo")

import numpy as np
import ml_dtypes

BF16 = ml_dtypes.bfloat16

B, L, D, T = 2, 2048, 1536, 2
NCORES = 8
CPB = NCORES // B     # cores per batch item = 4
NK = D // 128         # contraction k-tiles = 12
NJ = 512              # j columns per full matmul (one PSUM bank of fp32)
NNB = L // NJ         # j chunks = 4
NS = 4                # stationary row slots per core (128 rows each)

PHASES = [3, 2, 1, 0]           # chunk/phase order (heaviest work first)
PREPS = [0, 1, 2, 3]            # slot-window arrival + prep order
# DMA k-parts per chunk: chunk 3 (consumed first) is quartered so the
# first matmuls start on a quarter-chunk; finer splits lose more to the
# ~0.7us fixed issue cost per DMA than the earlier starts gain.
PARTS = {3: 4, 2: 2, 1: 2, 0: 2}


def row_of(s, cc):
    """Global 128-row block held by slot s on a core with variant cc."""
    return 4 * s + (s + cc) % 4


def groups_of(cc):
    """Schedule (shared shape, variant-dependent geometry): list of
    (I, colstart, F) in emission order; 20 groups = [statics (s<v) t0, t1,
    then partial t0, t1] per chunk phase v, phases in PHASES order.  The
    partial goes last: the statics only need the early slot windows, so
    the phase's bulk is never gated on the late-arriving window of its own
    slot."""
    gs = []
    for v in PHASES:
        q = (v + cc) % 4
        for s in range(v):
            for _t in range(T):
                gs.append((row_of(s, cc), NJ * v, NJ))
        for _t in range(T):
            gs.append((row_of(v, cc), NJ * v + 128 * q, NJ - 128 * q))
    return gs


NG = len(groups_of(0))    # 20

_CACHE = {}


def _get_nc():
    if "nc" in _CACHE:
        return _CACHE["nc"]
    import concourse.tile as tile
    from concourse.tile_rust import add_dep_helper
    from concourse import bacc, mybir

    f32, bf16 = mybir.dt.float32, mybir.dt.bfloat16
    nc = bacc.Bacc("TRN2", target_bir_lowering=False, debug=False,
                   num_devices=NCORES, enable_partition_id=True,
                   enable_asserts=False)
    ht_d = nc.dram_tensor("ht", [D, L], bf16, kind="ExternalInput")
    hw_d = nc.dram_tensor("hw", [D, NS * 128], bf16, kind="ExternalInput")
    wp_d = nc.dram_tensor("wp", [128, T * NK], bf16, kind="ExternalInput")
    out_d = nc.dram_tensor("out", [NG, 128, NJ], bf16, kind="ExternalOutput")

    with tile.TileContext(nc) as tc:
        with tc.tile_pool(name="big", bufs=1) as big, \
             tc.tile_pool(name="st", bufs=4) as stp, \
             tc.tile_pool(name="ps", bufs=4, space="PSUM") as psp, \
             tc.tile_pool(name="psw", bufs=1, space="PSUM") as psw:
            # wt[p, t*NK+k] = Wp[t, 128k+p] (pre-gathered on the host)
            wt = big.tile([128, T * NK], bf16, name="wt")
            # hst[p, s, t, k, r] = hw[128k+p, 128s+r] * Wp[t, 128k+p]
            hst = big.tile([128, NS, T, NK, 128], bf16, name="hst")
            # htw[p, k, 128s+r] = hw[128k+p, 128s+r]  (slot windows)
            htw = big.tile([128, NK, NS * 128], bf16, name="htw")
            # htall[p, k, j] = ht[128k+p, j]  (canonical, un-rolled)
            htall = big.tile([128, NK, L], bf16, name="htall")

            # Partition-id register load costs ~1.4us of queue time — issue
            # it first so it overlaps the framework preamble and DMA issues
            # instead of delaying the branch evaluations later.
            pid = nc.tensor.partition_id()
            cc = pid % 4

            nc.scalar.dma_start(wt[:], wp_d.ap())
            # Warm the PE clock (HAM un-throttles after ~3.4 us of activity)
            # with throwaway matmuls on a locally-initialized scratch tile —
            # no DMA dependency, so warmup starts during the preamble.  The
            # memset rides gpsimd so the vector queue stays clear for the
            # stationary preps.
            wdum = big.tile([128, NJ], bf16, name="wdum")
            nc.gpsimd.memset(wdum[:], 0.0)
            wacc = psw.tile([128, NJ], f32, name="wacc")
            for _ in range(16):
                nc.tensor.matmul(wacc[:, 0:128], wdum[:, 0:128],
                                 wdum[:, 0:128], start=True, stop=True)

            # Input stream, strictly ordered on the sync ring in consumption
            # order: slot-3 window, chunk 3, remaining slot windows, then
            # chunks 2, 1, 0.
            prev = None

            def chain(dma):
                nonlocal prev
                if prev is not None:
                    add_dep_helper(dma.ins, prev.ins, sync=False,
                                   reason="input stream in consumption order")
                prev = dma

            def win_dma(s):
                chain(nc.sync.dma_start(
                    htw[:, :, 128 * s:128 * (s + 1)],
                    hw_d.ap()[:, 128 * s:128 * (s + 1)]
                    .rearrange("(k p) r -> p k r", p=128)))

            def chunk_part(v, h, kq):
                chain(nc.sync.dma_start(
                    htall[:, h * kq:(h + 1) * kq, v * NJ:(v + 1) * NJ],
                    ht_d.ap()[h * kq * 128:(h + 1) * kq * 128,
                              v * NJ:(v + 1) * NJ]
                    .rearrange("(k p) j -> p k j", p=128)))

            # Slot-0 window first (gates the first static matmul), then all
            # of chunk 3 (it paces the whole first phase), then the
            # remaining slot windows (consumed in s order; slot v's own
            # window isn't needed until the end of phase v), then chunks
            # 2, 1, 0.
            win_dma(PREPS[0])
            for h in range(PARTS[3]):
                chunk_part(3, h, NK // PARTS[3])
            for s in PREPS[1:]:
                win_dma(s)
            for v in PHASES[1:]:
                for h in range(PARTS[v]):
                    chunk_part(v, h, NK // PARTS[v])

            # Stationary prep (straight-line; per-core rows arrive via hw):
            # hst[:, s, t] = htw window s  *  Wp[t] broadcast along rows.
            for s in PREPS:
                for t in range(T):
                    scale = (wt[:, t * NK:(t + 1) * NK].unsqueeze(2)
                             .broadcast_to([128, NK, 128]))
                    nc.vector.tensor_mul(
                        hst[:, s, t], htw[:, :, 128 * s:128 * (s + 1)],
                        scale)

            def emit_partial(v, accs, ccv):
                q = (v + ccv) % 4
                off, fw = NJ * v + 128 * q, NJ - 128 * q
                for t in range(T):
                    for k in range(NK):
                        nc.tensor.matmul(
                            accs[t][:, 0:fw], hst[:, v, t, k],
                            htall[:, k, off:off + fw],
                            start=(k == 0), stop=(k == NK - 1))

            def emit_static(s, v, acc, t):
                for k in range(NK):
                    nc.tensor.matmul(
                        acc[:], hst[:, s, t, k],
                        htall[:, k, v * NJ:(v + 1) * NJ],
                        start=(k == 0), stop=(k == NK - 1))

            def store(gi, acc):
                # Copies on scalar: the vector queue must stay free for the
                # stationary preps (a copy would trap a later prep behind a
                # matmul completion).  The last two groups (the v=0
                # partials) route around gpsimd, whose teardown drain is
                # ~2.7us: t0's chain hides inside t1's matmuls; t1's store
                # splits into two halves on parallel vector+sync /
                # scalar+scalar chains (sync's drain is ~8ns).
                if gi == NG - 2:
                    st = stp.tile([128, NJ], bf16, name="st", tag="st")
                    nc.vector.tensor_copy(st[:], acc[:])
                    nc.sync.dma_start(out_d.ap()[gi], st[:])
                elif gi == NG - 1:
                    for hh in range(2):
                        sth = stp.tile([128, NJ // 2], bf16, name="sth",
                                       tag="sth")
                        half = slice(hh * (NJ // 2), (hh + 1) * (NJ // 2))
                        if hh == 0:
                            nc.scalar.copy(sth[:], acc[:, half])
                            nc.scalar.dma_start(out_d.ap()[gi][:, half],
                                                sth[:])
                        else:
                            nc.vector.tensor_copy(sth[:], acc[:, half])
                            nc.sync.dma_start(out_d.ap()[gi][:, half],
                                              sth[:])
                else:
                    st = stp.tile([128, NJ], bf16, name="st", tag="st")
                    nc.scalar.copy(st[:], acc[:])
                    nc.gpsimd.dma_start(out_d.ap()[gi], st[:])

            gi = 0
            for v in PHASES:
                first_s = 0
                if v > 0:
                    # Each phase's opening static pair can be paced by its
                    # chunk's stream: one group consumes a k-part faster
                    # than parts arrive.  Interleaving t0/t1 by k-part (two
                    # PSUM banks accumulating concurrently) matches
                    # consumption to arrival and closes the per-part gaps —
                    # free when the data is already resident, and it keeps
                    # later phases gap-free when contention slows the
                    # stream.
                    accs0 = [psp.tile([128, NJ], f32, name="acc", tag="acc")
                             for _t in range(T)]
                    kq = NK // PARTS[v]
                    for h in range(PARTS[v]):
                        for t in range(T):
                            for k in range(h * kq, (h + 1) * kq):
                                nc.tensor.matmul(
                                    accs0[t][:], hst[:, 0, t, k],
                                    htall[:, k, v * NJ:(v + 1) * NJ],
                                    start=(k == 0), stop=(k == NK - 1))
                    for t in range(T):
                        store(gi, accs0[t])
                        gi += 1
                    first_s = 1
                for s in range(first_s, v):
                    for t in range(T):
                        acc = psp.tile([128, NJ], f32, name="acc", tag="acc")
                        emit_static(s, v, acc, t)
                        store(gi, acc)
                        gi += 1
                accs = [psp.tile([128, NJ], f32, name="acc", tag="acc")
                        for _t in range(T)]
                with tc.If(cc <= 1) as c1:
                    with tc.If(cc == 0) as c2:
                        emit_partial(v, accs, 0)
                    with c2.Else():
                        emit_partial(v, accs, 1)
                with c1.Else():
                    with tc.If(cc == 2) as c3:
                        emit_partial(v, accs, 2)
                    with c3.Else():
                        emit_partial(v, accs, 3)
                for t in range(T):
                    store(gi, accs[t])
                    gi += 1
    nc.compile()
    _CACHE["nc"] = nc
    return nc


def make_in_maps(h, W):
    # wp[p, t*NK+k] = Wp[t, 128k+p]
    wp = np.ascontiguousarray(
        W[:, :D].reshape(T, NK, 128).transpose(2, 0, 1)
        .reshape(128, T * NK)).astype(BF16)
    hts = [np.ascontiguousarray(h[bi].T).astype(BF16) for bi in range(B)]
    in_maps = []
    for c in range(NCORES):
        bi, cc = c // CPB, c % CPB
        hw = np.concatenate(
            [hts[bi][:, 128 * row_of(s, cc):128 * row_of(s, cc) + 128]
             for s in range(NS)], axis=1)
        in_maps.append({"ht": hts[bi], "hw": np.ascontiguousarray(hw),
                        "wp": wp})
    return in_maps


def kernel(hidden_states, W, b):
    from concourse.bass_utils import run_bass_kernel_spmd

    h = np.ascontiguousarray(hidden_states, dtype=np.float32)
    W = np.asarray(W, dtype=np.float32)
    bias = np.asarray(b, dtype=np.float32)
    nc = _get_nc()

    res = run_bass_kernel_spmd(nc, make_in_maps(h, W),
                               core_ids=list(range(NCORES)))
    full = np.empty((B, L, L, T), np.float32)
    for c in range(NCORES):
        bi, cc = c // CPB, c % CPB
        blocks = np.asarray(res.results[c]["out"]).astype(np.float32)
        for gi, (I, colstart, fw) in enumerate(groups_of(cc)):
            t = gi % T
            rows = slice(128 * I, 128 * I + 128)
            full[bi, rows, colstart:colstart + fw, t] = blocks[gi, :, 0:fw]
    # Mirror: keep computed j >= i, take j < i from the transpose.
    idx = np.arange(L)
    mask = (idx[None, :] >= idx[:, None])[None, :, :, None]
    out = np.where(mask, full, full.transpose(0, 2, 1, 3))
    if np.any(bias != 0):
        out += bias
    return out
